# revision 19
# baseline (speedup 1.0000x reference)
"""DKVMN (DeepIRT) forward pass on 8 Trainium2 NeuronCores.

Strategy (v3)
-------------
Pure data parallel over the batch (2048 -> 256 per core, 2 partition-tiles
of 128). Token-dependent quantities are folded into host-precomputed gather
tables (weight-only preprocessing):

  wh[q]  = [ w | hq ]          w = softmax(q_embed @ key_memory^T), hq = q-part of MLP
  ea[qa] = [ -e | a | -1/e ]   e = sigmoid(qa_emb@We+be), a = tanh(qa_emb@Wa+ba)

Per step, per 128-row tile, the Scalar engine (ACT) builds the combined
rank-1 tensors with 50 per-slot scaled copies (scale = w[b,m] as a
per-partition scalar):

  GX[:, m, 0:2V] = [-e | a] * w[b,m]     ->  G' = -w(x)e,  X = w(x)a

and the Vector engine (DVE) runs only three full-state passes plus a small
reduction tree -- all in 2x fp16 mode:

  P  = Mv * G'            (erase term, negated)
  Mv = Mv + P             (apply erase)
  Mv = Mv + X             (apply add)
  tree(P) -> -e*read ; read = tree(P) * (-1/e)   (read falls out of P!)

The read identity: sum_m P[b,m,v] = -e[b,v] * sum_m w[b,m]*Mv[b,m,v], so one
tiny multiply by the tabulated -1/e recovers the read vector -- no separate
Mv*w_rep pass and no w replication at all.

GPSIMD is banned from elementwise work (its SBUF port is shared with the
DVE: a GPSIMD tensor op stalls the DVE for ~20us -- measured); it only
generates SWDGE gather descriptors.

The prediction MLP is decoupled from the scan: per step the PE computes
z = read @ W1r (transpose + 2 matmuls into PSUM), z and hq stream to HBM,
and one batched tail pass (add, tanh, *w2, reduce, sigmoid) produces all
preds at the end.
"""

import os
import sys

for _p in ("/root/.axon_site/_ro/trn_rl_repo", "/opt/trn_rl_repo"):
    if os.path.isdir(_p) and _p not in sys.path:
        sys.path.append(_p)

import numpy as np

import concourse.bacc as bacc
import concourse.bass as bass
import concourse.tile as tile
from concourse import mybir
from concourse.bass_utils import run_bass_kernel_spmd
from concourse.masks import make_identity

# Problem shapes (hardcoded per harness contract)
B, S, M, V, KD, FC = 2048, 200, 50, 200, 50, 50
NQ, NQA = 5001, 10001
NCORES = 8
BL = B // NCORES      # 256 batch rows per core
P = 128               # SBUF partitions
NT = BL // P          # 2 batch tiles per core
KSTEPS = 2            # time steps per gather block
NBLK = S // KSTEPS
EAW = 640             # ea-table row: [-e(200) | a(200) | -1/e(200) | pad] = 1280B
WHW = 128             # wh-table row: [w(50) | pad | hq(50) | pad] = 256B
IDX_PER_BLK = BL * KSTEPS        # 512 gathered rows per block per table
IDXCOLS = BL * S // 16           # wrapped idx array columns
ZW = NT * KSTEPS * FC            # z/hq row elems per block (200)
MJ = 12                          # slot builds m < MJ go to DVE, rest to ACT

_prog_cache = {}


def _build_program(steps=S):
    dt = mybir.dt
    nc = bacc.Bacc("TRN2", debug=False)

    ea_t = nc.dram_tensor("ea_table", [NQA, EAW], dt.float16, kind="ExternalInput")
    wh_t = nc.dram_tensor("wh_table", [NQ, WHW], dt.float16, kind="ExternalInput")
    w1r_d = nc.dram_tensor("w1r", [2, 100, FC], dt.float16, kind="ExternalInput")
    w2_d = nc.dram_tensor("w2rep", [P, FC], dt.float16, kind="ExternalInput")
    b2_d = nc.dram_tensor("b2rep", [P, 1], dt.float32, kind="ExternalInput")
    mv_d = nc.dram_tensor("mv_init", [1, M * V], dt.float16, kind="ExternalInput")
    qi_d = nc.dram_tensor("qidx", [P, IDXCOLS], dt.int16, kind="ExternalInput")
    qa_d = nc.dram_tensor("qaidx", [P, IDXCOLS], dt.int16, kind="ExternalInput")
    preds_d = nc.dram_tensor("preds_out", [BL, S], dt.float32, kind="ExternalOutput")
    zb_d = nc.dram_tensor("zbuf", [NBLK, P, ZW], dt.float16, kind="Internal")
    hq_d = nc.dram_tensor("hqbuf", [NBLK, P, ZW], dt.float16, kind="Internal")

    nblk = steps // KSTEPS

    from contextlib import ExitStack

    mult = mybir.AluOpType.mult
    addop = mybir.AluOpType.add
    COPY = mybir.ActivationFunctionType.Copy

    with tile.TileContext(nc) as tc:
        with ExitStack() as ctx:
            consts = ctx.enter_context(tc.tile_pool(name="consts", bufs=1))
            state = ctx.enter_context(tc.tile_pool(name="state", bufs=1))
            gath = ctx.enter_context(tc.tile_pool(name="gath", bufs=2))
            small = ctx.enter_context(tc.tile_pool(name="small", bufs=3))
            psum = ctx.enter_context(tc.tile_pool(name="psum", bufs=2, space="PSUM"))

            # ---- constants ----
            w1r_sb = consts.tile([100, 2, FC], dt.float16)
            for c in range(2):
                nc.sync.dma_start(out=w1r_sb[:, c, :], in_=w1r_d[c])
            ident = consts.tile([P, P], dt.float16)
            make_identity(nc, ident)

            # ---- persistent state ----
            Mvs, Ps, GXs = [], [], []
            for tl in range(NT):
                Mv = state.tile([P, M, V], dt.float16, tag=f"mv{tl}", name=f"mv{tl}")
                nc.sync.dma_start(
                    out=Mv[:].rearrange("p m v -> p (m v)"),
                    in_=mv_d[:].to_broadcast((P, M * V)),
                )
                Mvs.append(Mv)
                Ps.append(state.tile([P, M, V], dt.float16, tag=f"pp{tl}", name=f"pp{tl}"))
                GXs.append(state.tile([P, M, 2 * V], dt.float16, tag=f"gx{tl}", name=f"gx{tl}"))

            # ---- scan ----
            pending = None
            for g in range(nblk):
                qi = gath.tile([P, IDX_PER_BLK // 16], dt.int16, tag="qi")
                qa = gath.tile([P, IDX_PER_BLK // 16], dt.int16, tag="qa")
                c0 = g * (IDX_PER_BLK // 16)
                nc.sync.dma_start(out=qi[:], in_=qi_d[:, c0:c0 + IDX_PER_BLK // 16])
                nc.sync.dma_start(out=qa[:], in_=qa_d[:, c0:c0 + IDX_PER_BLK // 16])
                ea_blk = gath.tile([P, NT * KSTEPS, EAW], dt.float16, tag="ea")
                wh_blk = gath.tile([P, NT * KSTEPS, WHW], dt.float16, tag="wh")
                nc.gpsimd.dma_gather(ea_blk[:], ea_t[:], qa[:], IDX_PER_BLK, IDX_PER_BLK, EAW)
                nc.gpsimd.dma_gather(wh_blk[:], wh_t[:], qi[:], IDX_PER_BLK, IDX_PER_BLK, WHW)
                zstage = gath.tile([P, NT, KSTEPS, FC], dt.float16, tag="zst")
                # fp32 copy of the w columns (ACT scale APs must be fp32)
                wf = gath.tile([P, NT * KSTEPS, M], dt.float32, tag="wf")
                nc.vector.tensor_copy(wf[:], wh_blk[:, :, 0:M])

                def z_pipeline(ea_ref, zst, c, tl, k, Pt_):
                    # read = tree(P) * (-1/e), then z = read @ W1r via PE.
                    # Deferred one tile-step so the ACT queue (readT copies)
                    # and DVE queue (zcopy) never block on the PE round-trip.
                    def emit():
                        read = small.tile([P, V], dt.float16, tag="read")
                        nc.vector.tensor_mul(read[:], Pt_[:, 0, :], ea_ref[:, c, 2 * V:3 * V])
                        readT = small.tile([100, 2, P], dt.float16, tag="readT")
                        for cc in range(2):
                            pT = psum.tile([100, P], dt.float16, tag="pT")
                            nc.tensor.transpose(pT[:], read[:, cc * 100:(cc + 1) * 100], ident[:])
                            nc.scalar.copy(readT[:, cc, :], pT[:])
                        h_ps = psum.tile([P, FC], dt.float32, tag="hps")
                        nc.tensor.matmul(h_ps[:], lhsT=readT[:, 0, :], rhs=w1r_sb[:, 0, :],
                                         start=True, stop=False)
                        nc.tensor.matmul(h_ps[:], lhsT=readT[:, 1, :], rhs=w1r_sb[:, 1, :],
                                         start=False, stop=True)
                        nc.vector.tensor_copy(zst[:, tl, k, :], h_ps[:])
                    return emit

                for k in range(KSTEPS):
                    for tl in range(NT):
                        c = k * NT + tl
                        Mv, Pt, GX = Mvs[tl], Ps[tl], GXs[tl]

                        # GX[:, m, :] = [-e | a] * w[b, m]  (50 slot ops,
                        # split DVE/ACT to balance the engines)
                        for m in range(MJ):
                            nc.vector.tensor_scalar_mul(
                                GX[:, m, :], ea_blk[:, c, 0:2 * V], wf[:, c, m:m + 1],
                            )
                        for m in range(MJ, M):
                            nc.scalar.activation(
                                GX[:, m, :], ea_blk[:, c, 0:2 * V], COPY,
                                scale=wf[:, c, m:m + 1],
                            )
                        if pending is not None:
                            pending()
                            pending = None

                        # DVE: three full passes, all dense fp16 2x
                        nc.vector.tensor_mul(Pt[:], Mv[:], GX[:, :, 0:V])
                        nc.vector.tensor_add(Mv[:], Mv[:], Pt[:])
                        nc.vector.tensor_add(Mv[:], Mv[:], GX[:, :, V:2 * V])

                        # DVE: add-tree over m on Pt (in place)
                        nc.vector.tensor_add(Pt[:, 0:25, :], Pt[:, 0:25, :], Pt[:, 25:50, :])
                        nc.vector.tensor_add(Pt[:, 0:12, :], Pt[:, 0:12, :], Pt[:, 12:24, :])
                        nc.vector.tensor_add(Pt[:, 0:6, :], Pt[:, 0:6, :], Pt[:, 6:12, :])
                        nc.vector.tensor_add(Pt[:, 0:3, :], Pt[:, 0:3, :], Pt[:, 3:6, :])
                        nc.vector.tensor_add(Pt[:, 0:1, :], Pt[:, 0:1, :], Pt[:, 1:2, :])
                        nc.vector.tensor_add(Pt[:, 0:1, :], Pt[:, 0:1, :], Pt[:, 2:3, :])
                        nc.vector.tensor_add(Pt[:, 0:1, :], Pt[:, 0:1, :], Pt[:, 24:25, :])

                        pending = z_pipeline(ea_blk, zstage, c, tl, k, Pt)

                # flush the last tile-step's z pipeline before the block DMA
                if pending is not None:
                    pending()
                    pending = None

                # per-block: stream z and hq to HBM
                nc.sync.dma_start(out=zb_d[g], in_=zstage[:].rearrange("p n k f -> p (n k f)"))
                hq_view = wh_blk[:].rearrange("p (k n) w -> p k n w", n=NT)
                for tl in range(NT):
                    nc.sync.dma_start(
                        out=hq_d[g].rearrange("p (n k f) -> p n k f", n=NT, k=KSTEPS)[:, tl],
                        in_=hq_view[:, :, tl, 64:64 + FC],
                    )

        # ---- batched MLP tail ----
        with ExitStack() as ctx:
            tconsts = ctx.enter_context(tc.tile_pool(name="tconsts", bufs=1))
            tpool = ctx.enter_context(tc.tile_pool(name="tail", bufs=2))
            w2_sb = tconsts.tile([P, FC], dt.float16)
            nc.sync.dma_start(out=w2_sb[:], in_=w2_d[:])
            b2_sb = tconsts.tile([P, 1], dt.float32)
            nc.sync.dma_start(out=b2_sb[:], in_=b2_d[:])
            pv = preds_d[:].rearrange("(n p) s -> n p s", p=P)

            CH = 25
            g0 = 0
            while g0 < nblk:
                ch = min(CH, nblk - g0)
                nrow = ch * NT * KSTEPS
                zt = tpool.tile([P, ch, ZW], dt.float16, tag="zt")
                hqt = tpool.tile([P, ch, ZW], dt.float16, tag="hqt")
                nc.sync.dma_start(out=zt[:], in_=zb_d[g0:g0 + ch].rearrange("g p x -> p g x"))
                nc.sync.dma_start(out=hqt[:], in_=hq_d[g0:g0 + ch].rearrange("g p x -> p g x"))
                hpre = tpool.tile([P, ch * ZW], dt.float16, tag="hpre")
                nc.vector.tensor_add(hpre[:], zt[:].rearrange("p g x -> p (g x)"),
                                     hqt[:].rearrange("p g x -> p (g x)"))
                hact = tpool.tile([P, nrow, FC], dt.float16, tag="hact")
                nc.scalar.activation(hact[:].rearrange("p r f -> p (r f)"), hpre[:],
                                     mybir.ActivationFunctionType.Tanh)
                hw2 = tpool.tile([P, nrow, FC], dt.float16, tag="hw2")
                nc.vector.tensor_mul(hw2[:], hact[:],
                                     w2_sb[:, None, :].to_broadcast((P, nrow, FC)))
                pacc = tpool.tile([P, nrow], dt.float32, tag="pacc")
                nc.vector.tensor_reduce(pacc[:], hw2[:], mybir.AxisListType.X, addop)
                psig = tpool.tile([P, ch, NT, KSTEPS], dt.float32, tag="psig")
                nc.scalar.activation(
                    psig[:].rearrange("p g n k -> p (g n k)"), pacc[:],
                    mybir.ActivationFunctionType.Sigmoid, bias=b2_sb[:],
                )
                for tl in range(NT):
                    nc.sync.dma_start(
                        out=pv[tl][:, g0 * KSTEPS:(g0 + ch) * KSTEPS].rearrange(
                            "p (g k) -> p g k", k=KSTEPS),
                        in_=psig[:, :, tl, :],
                    )
                g0 += ch

    nc.finalize()
    return nc


def _wrap_idx(seq):
    """seq [N] -> [128, N//16] int16 wrapped (idx i at [i%16, i//16], 8x replicated)."""
    n = seq.shape[0]
    arr16 = seq.reshape(n // 16, 16).T.astype(np.int16)
    return np.tile(arr16, (8, 1))


def _host_tables(inputs):
    f32 = np.float32
    qe = inputs["q_embed_w"].astype(f32)
    qae = inputs["qa_embed_w"].astype(f32)
    km = inputs["key_memory"].astype(f32)

    logits = qe @ km.T
    ex = np.exp(logits - logits.max(-1, keepdims=True))
    wsoft = ex / ex.sum(-1, keepdims=True)
    hq = qe @ inputs["pred_w1"][V:, :].astype(f32) + inputs["pred_b1"].astype(f32)
    esig = 1.0 / (1.0 + np.exp(-(qae @ inputs["erase_w"].astype(f32) + inputs["erase_b"].astype(f32))))
    atanh = np.tanh(qae @ inputs["add_w"].astype(f32) + inputs["add_b"].astype(f32))

    ea = np.zeros((NQA, EAW), np.float16)
    ea[:, 0:V] = (-esig).astype(np.float16)
    ea[:, V:2 * V] = atanh.astype(np.float16)
    ea[:, 2 * V:3 * V] = (-1.0 / esig).astype(np.float16)
    wh = np.zeros((NQ, WHW), np.float16)
    wh[:, 0:M] = wsoft.astype(np.float16)
    wh[:, 64:64 + FC] = hq.astype(np.float16)

    w1r = inputs["pred_w1"][:V, :].astype(np.float16).reshape(2, 100, FC)
    w2rep = np.tile(inputs["pred_w2"][:, 0].astype(np.float16)[None, :], (P, 1))
    b2rep = np.full((P, 1), inputs["pred_b2"][0], np.float32)
    mv_init = inputs["init_value_memory"].astype(np.float16).reshape(1, -1)
    return dict(ea_table=ea, wh_table=wh, w1r=w1r, w2rep=w2rep, b2rep=b2rep,
                mv_init=mv_init)


def kernel(**inputs):
    inputs = {k: np.asarray(v) for k, v in inputs.items()}
    steps = int(os.environ.get("KERNEL_STEPS", S))

    if steps not in _prog_cache:
        _prog_cache[steps] = _build_program(steps)
    nc = _prog_cache[steps]

    shared = _host_tables(inputs)
    q = inputs["q_data"].astype(np.int64)
    qa = inputs["qa_data"].astype(np.int64)

    in_maps = []
    for core in range(NCORES):
        qs = q[core * BL:(core + 1) * BL]       # [256, S]
        qas = qa[core * BL:(core + 1) * BL]
        # gather order: block g, step k, tile tl, partition p
        #   -> element (g*K + k) of column (tl*128+p)
        def order(x):
            # x [BL, S] -> [S, NT, P] -> [NBLK, KSTEPS, NT, P] flat
            xt = x.T.reshape(S, NT, P)
            return xt.reshape(NBLK, KSTEPS, NT, P).reshape(-1)
        m = dict(shared)
        m["qidx"] = _wrap_idx(order(qs))
        m["qaidx"] = _wrap_idx(order(qas))
        in_maps.append(m)

    trace = bool(int(os.environ.get("KERNEL_TRACE", "0")))
    res = run_bass_kernel_spmd(nc, in_maps, core_ids=list(range(NCORES)), trace=trace)
    global LAST_RESULTS
    LAST_RESULTS = res
    preds = np.concatenate(
        [res.results[i]["preds_out"] for i in range(NCORES)], axis=0
    ).astype(np.float32)
    z = np.zeros_like(preds)
    return (preds, z, z, z)


# revision 21
# speedup vs baseline: 1.0680x; 1.0680x over previous
"""DKVMN (DeepIRT) forward pass on 8 Trainium2 NeuronCores.

Strategy (v3)
-------------
Pure data parallel over the batch (2048 -> 256 per core, 2 partition-tiles
of 128). Token-dependent quantities are folded into host-precomputed gather
tables (weight-only preprocessing):

  wh[q]  = [ w | hq ]          w = softmax(q_embed @ key_memory^T), hq = q-part of MLP
  ea[qa] = [ -e | a | -1/e ]   e = sigmoid(qa_emb@We+be), a = tanh(qa_emb@Wa+ba)

Per step, per 128-row tile, the Scalar engine (ACT) builds the combined
rank-1 tensors with 50 per-slot scaled copies (scale = w[b,m] as a
per-partition scalar):

  GX[:, m, 0:2V] = [-e | a] * w[b,m]     ->  G' = -w(x)e,  X = w(x)a

and the Vector engine (DVE) runs only three full-state passes plus a small
reduction tree -- all in 2x fp16 mode:

  P  = Mv * G'            (erase term, negated)
  Mv = Mv + P             (apply erase)
  Mv = Mv + X             (apply add)
  tree(P) -> -e*read ; read = tree(P) * (-1/e)   (read falls out of P!)

The read identity: sum_m P[b,m,v] = -e[b,v] * sum_m w[b,m]*Mv[b,m,v], so one
tiny multiply by the tabulated -1/e recovers the read vector -- no separate
Mv*w_rep pass and no w replication at all.

GPSIMD is banned from elementwise work (its SBUF port is shared with the
DVE: a GPSIMD tensor op stalls the DVE for ~20us -- measured); it only
generates SWDGE gather descriptors.

The prediction MLP is decoupled from the scan: per step the PE computes
z = read @ W1r (transpose + 2 matmuls into PSUM), z and hq stream to HBM,
and one batched tail pass (add, tanh, *w2, reduce, sigmoid) produces all
preds at the end.
"""

import os
import sys

for _p in ("/root/.axon_site/_ro/trn_rl_repo", "/opt/trn_rl_repo"):
    if os.path.isdir(_p) and _p not in sys.path:
        sys.path.append(_p)

import numpy as np

import concourse.bacc as bacc
import concourse.bass as bass
import concourse.tile as tile
from concourse import mybir
from concourse.bass_utils import run_bass_kernel_spmd
from concourse.masks import make_identity

# Problem shapes (hardcoded per harness contract)
B, S, M, V, KD, FC = 2048, 200, 50, 200, 50, 50
NQ, NQA = 5001, 10001
NCORES = 8
BL = B // NCORES      # 256 batch rows per core
P = 128               # SBUF partitions
NT = BL // P          # 2 batch tiles per core
KSTEPS = 2            # time steps per gather block
NBLK = S // KSTEPS
EAW = 640             # ea-table row: [-e(200) | a(200) | -1/e(200) | pad] = 1280B
WHW = 128             # wh-table row: [w(50) | pad | hq(50) | pad] = 256B
IDX_PER_BLK = BL * KSTEPS        # 512 gathered rows per block per table
IDXCOLS = BL * S // 16           # wrapped idx array columns
ZW = NT * KSTEPS * FC            # z/hq row elems per block (200)
MJ = 12                          # slot builds m < MJ go to DVE, rest to ACT

_prog_cache = {}


def _build_program(steps=S):
    dt = mybir.dt
    nc = bacc.Bacc("TRN2", debug=False)

    ea_t = nc.dram_tensor("ea_table", [NQA, EAW], dt.float16, kind="ExternalInput")
    wh_t = nc.dram_tensor("wh_table", [NQ, WHW], dt.float16, kind="ExternalInput")
    w1r_d = nc.dram_tensor("w1r", [2, 100, FC], dt.float16, kind="ExternalInput")
    w2_d = nc.dram_tensor("w2rep", [P, FC], dt.float16, kind="ExternalInput")
    b2_d = nc.dram_tensor("b2rep", [P, 1], dt.float32, kind="ExternalInput")
    mv_d = nc.dram_tensor("mv_init", [1, M * V], dt.float16, kind="ExternalInput")
    qi_d = nc.dram_tensor("qidx", [P, IDXCOLS], dt.int16, kind="ExternalInput")
    qa_d = nc.dram_tensor("qaidx", [P, IDXCOLS], dt.int16, kind="ExternalInput")
    preds_d = nc.dram_tensor("preds_out", [BL, S], dt.float32, kind="ExternalOutput")
    zb_d = nc.dram_tensor("zbuf", [NBLK, P, ZW], dt.float16, kind="Internal")
    hq_d = nc.dram_tensor("hqbuf", [NBLK, P, ZW], dt.float16, kind="Internal")

    nblk = steps // KSTEPS

    from contextlib import ExitStack

    mult = mybir.AluOpType.mult
    addop = mybir.AluOpType.add
    COPY = mybir.ActivationFunctionType.Copy

    with tile.TileContext(nc) as tc:
        with ExitStack() as ctx:
            consts = ctx.enter_context(tc.tile_pool(name="consts", bufs=1))
            state = ctx.enter_context(tc.tile_pool(name="state", bufs=1))
            gath = ctx.enter_context(tc.tile_pool(name="gath", bufs=2))
            small = ctx.enter_context(tc.tile_pool(name="small", bufs=3))
            psum = ctx.enter_context(tc.tile_pool(name="psum", bufs=2, space="PSUM"))

            # ---- constants ----
            w1r_sb = consts.tile([100, 2, FC], dt.float16)
            for c in range(2):
                nc.sync.dma_start(out=w1r_sb[:, c, :], in_=w1r_d[c])
            ident = consts.tile([P, P], dt.float16)
            make_identity(nc, ident)

            # ---- persistent state ----
            Mvs, Ps, GXs = [], [], []
            for tl in range(NT):
                Mv = state.tile([P, M, V], dt.float16, tag=f"mv{tl}", name=f"mv{tl}")
                nc.sync.dma_start(
                    out=Mv[:].rearrange("p m v -> p (m v)"),
                    in_=mv_d[:].to_broadcast((P, M * V)),
                )
                Mvs.append(Mv)
                Ps.append(state.tile([P, M, V], dt.float16, tag=f"pp{tl}", name=f"pp{tl}"))
                GXs.append(state.tile([P, M, 2 * V], dt.float16, tag=f"gx{tl}", name=f"gx{tl}"))

            # ---- scan ----
            pending = None
            for g in range(nblk):
                qi = gath.tile([P, IDX_PER_BLK // 16], dt.int16, tag="qi")
                qa = gath.tile([P, IDX_PER_BLK // 16], dt.int16, tag="qa")
                c0 = g * (IDX_PER_BLK // 16)
                nc.sync.dma_start(out=qi[:], in_=qi_d[:, c0:c0 + IDX_PER_BLK // 16])
                nc.sync.dma_start(out=qa[:], in_=qa_d[:, c0:c0 + IDX_PER_BLK // 16])
                ea_blk = gath.tile([P, NT * KSTEPS, EAW], dt.float16, tag="ea")
                wh_blk = gath.tile([P, NT * KSTEPS, WHW], dt.float16, tag="wh")
                nc.gpsimd.dma_gather(ea_blk[:], ea_t[:], qa[:], IDX_PER_BLK, IDX_PER_BLK, EAW)
                nc.gpsimd.dma_gather(wh_blk[:], wh_t[:], qi[:], IDX_PER_BLK, IDX_PER_BLK, WHW)
                zstage = gath.tile([P, NT, KSTEPS, FC], dt.float16, tag="zst")
                # fp32 copy of the w columns (ACT scale APs must be fp32)
                wf = gath.tile([P, NT * KSTEPS, M], dt.float32, tag="wf")
                nc.vector.tensor_copy(wf[:], wh_blk[:, :, 0:M])

                def z_pipeline(read_, zst, tl, k):
                    # z = read @ W1r via PE.  Deferred one tile-step so the
                    # ACT queue (readT/z copies) trails the PE round-trip by
                    # a full tile-step and never blocks the slot stream; the
                    # DVE queue stays free of PSUM waits entirely.
                    def emit():
                        readT = small.tile([100, 2, P], dt.float16, tag="readT")
                        for cc in range(2):
                            pT = psum.tile([100, P], dt.float16, tag="pT")
                            nc.tensor.transpose(pT[:], read_[:, cc * 100:(cc + 1) * 100], ident[:])
                            nc.scalar.copy(readT[:, cc, :], pT[:])
                        h_ps = psum.tile([P, FC], dt.float32, tag="hps")
                        nc.tensor.matmul(h_ps[:], lhsT=readT[:, 0, :], rhs=w1r_sb[:, 0, :],
                                         start=True, stop=False)
                        nc.tensor.matmul(h_ps[:], lhsT=readT[:, 1, :], rhs=w1r_sb[:, 1, :],
                                         start=False, stop=True)
                        nc.scalar.activation(zst[:, tl, k, :], h_ps[:], COPY)
                    return emit

                for k in range(KSTEPS):
                    for tl in range(NT):
                        c = k * NT + tl
                        Mv, Pt, GX = Mvs[tl], Ps[tl], GXs[tl]

                        # GX[:, m, :] = [-e | a] * w[b, m]  (50 slot ops,
                        # split DVE/ACT to balance the engines)
                        for m in range(MJ):
                            nc.vector.tensor_scalar_mul(
                                GX[:, m, :], ea_blk[:, c, 0:2 * V], wf[:, c, m:m + 1],
                            )
                        for m in range(MJ, M):
                            nc.scalar.activation(
                                GX[:, m, :], ea_blk[:, c, 0:2 * V], COPY,
                                scale=wf[:, c, m:m + 1],
                            )
                        if pending is not None:
                            pending()
                            pending = None

                        # DVE: three full passes, all dense fp16 2x
                        nc.vector.tensor_mul(Pt[:], Mv[:], GX[:, :, 0:V])
                        nc.vector.tensor_add(Mv[:], Mv[:], Pt[:])
                        nc.vector.tensor_add(Mv[:], Mv[:], GX[:, :, V:2 * V])

                        # DVE: add-tree over m on Pt (in place)
                        nc.vector.tensor_add(Pt[:, 0:25, :], Pt[:, 0:25, :], Pt[:, 25:50, :])
                        nc.vector.tensor_add(Pt[:, 0:12, :], Pt[:, 0:12, :], Pt[:, 12:24, :])
                        nc.vector.tensor_add(Pt[:, 0:6, :], Pt[:, 0:6, :], Pt[:, 6:12, :])
                        nc.vector.tensor_add(Pt[:, 0:3, :], Pt[:, 0:3, :], Pt[:, 3:6, :])
                        nc.vector.tensor_add(Pt[:, 0:1, :], Pt[:, 0:1, :], Pt[:, 1:2, :])
                        nc.vector.tensor_add(Pt[:, 0:1, :], Pt[:, 0:1, :], Pt[:, 2:3, :])
                        nc.vector.tensor_add(Pt[:, 0:1, :], Pt[:, 0:1, :], Pt[:, 24:25, :])
                        # read = tree(P) * (-1/e)
                        read = small.tile([P, V], dt.float16, tag="read")
                        nc.vector.tensor_mul(read[:], Pt[:, 0, :], ea_blk[:, c, 2 * V:3 * V])

                        pending = z_pipeline(read, zstage, tl, k)

                # flush the last tile-step's z pipeline before the block DMA
                if pending is not None:
                    pending()
                    pending = None

                # per-block: stream z and hq to HBM
                nc.sync.dma_start(out=zb_d[g], in_=zstage[:].rearrange("p n k f -> p (n k f)"))
                hq_view = wh_blk[:].rearrange("p (k n) w -> p k n w", n=NT)
                for tl in range(NT):
                    nc.sync.dma_start(
                        out=hq_d[g].rearrange("p (n k f) -> p n k f", n=NT, k=KSTEPS)[:, tl],
                        in_=hq_view[:, :, tl, 64:64 + FC],
                    )

        # ---- batched MLP tail ----
        with ExitStack() as ctx:
            tconsts = ctx.enter_context(tc.tile_pool(name="tconsts", bufs=1))
            tpool = ctx.enter_context(tc.tile_pool(name="tail", bufs=2))
            w2_sb = tconsts.tile([P, FC], dt.float16)
            nc.sync.dma_start(out=w2_sb[:], in_=w2_d[:])
            b2_sb = tconsts.tile([P, 1], dt.float32)
            nc.sync.dma_start(out=b2_sb[:], in_=b2_d[:])
            pv = preds_d[:].rearrange("(n p) s -> n p s", p=P)

            CH = 25
            g0 = 0
            while g0 < nblk:
                ch = min(CH, nblk - g0)
                nrow = ch * NT * KSTEPS
                zt = tpool.tile([P, ch, ZW], dt.float16, tag="zt")
                hqt = tpool.tile([P, ch, ZW], dt.float16, tag="hqt")
                nc.sync.dma_start(out=zt[:], in_=zb_d[g0:g0 + ch].rearrange("g p x -> p g x"))
                nc.sync.dma_start(out=hqt[:], in_=hq_d[g0:g0 + ch].rearrange("g p x -> p g x"))
                hpre = tpool.tile([P, ch * ZW], dt.float16, tag="hpre")
                nc.vector.tensor_add(hpre[:], zt[:].rearrange("p g x -> p (g x)"),
                                     hqt[:].rearrange("p g x -> p (g x)"))
                hact = tpool.tile([P, nrow, FC], dt.float16, tag="hact")
                nc.scalar.activation(hact[:].rearrange("p r f -> p (r f)"), hpre[:],
                                     mybir.ActivationFunctionType.Tanh)
                hw2 = tpool.tile([P, nrow, FC], dt.float16, tag="hw2")
                nc.vector.tensor_mul(hw2[:], hact[:],
                                     w2_sb[:, None, :].to_broadcast((P, nrow, FC)))
                pacc = tpool.tile([P, nrow], dt.float32, tag="pacc")
                nc.vector.tensor_reduce(pacc[:], hw2[:], mybir.AxisListType.X, addop)
                psig = tpool.tile([P, ch, NT, KSTEPS], dt.float32, tag="psig")
                nc.scalar.activation(
                    psig[:].rearrange("p g n k -> p (g n k)"), pacc[:],
                    mybir.ActivationFunctionType.Sigmoid, bias=b2_sb[:],
                )
                for tl in range(NT):
                    nc.sync.dma_start(
                        out=pv[tl][:, g0 * KSTEPS:(g0 + ch) * KSTEPS].rearrange(
                            "p (g k) -> p g k", k=KSTEPS),
                        in_=psig[:, :, tl, :],
                    )
                g0 += ch

    nc.finalize()
    return nc


def _wrap_idx(seq):
    """seq [N] -> [128, N//16] int16 wrapped (idx i at [i%16, i//16], 8x replicated)."""
    n = seq.shape[0]
    arr16 = seq.reshape(n // 16, 16).T.astype(np.int16)
    return np.tile(arr16, (8, 1))


def _host_tables(inputs):
    f32 = np.float32
    qe = inputs["q_embed_w"].astype(f32)
    qae = inputs["qa_embed_w"].astype(f32)
    km = inputs["key_memory"].astype(f32)

    logits = qe @ km.T
    ex = np.exp(logits - logits.max(-1, keepdims=True))
    wsoft = ex / ex.sum(-1, keepdims=True)
    hq = qe @ inputs["pred_w1"][V:, :].astype(f32) + inputs["pred_b1"].astype(f32)
    esig = 1.0 / (1.0 + np.exp(-(qae @ inputs["erase_w"].astype(f32) + inputs["erase_b"].astype(f32))))
    atanh = np.tanh(qae @ inputs["add_w"].astype(f32) + inputs["add_b"].astype(f32))

    ea = np.zeros((NQA, EAW), np.float16)
    ea[:, 0:V] = (-esig).astype(np.float16)
    ea[:, V:2 * V] = atanh.astype(np.float16)
    ea[:, 2 * V:3 * V] = (-1.0 / esig).astype(np.float16)
    wh = np.zeros((NQ, WHW), np.float16)
    wh[:, 0:M] = wsoft.astype(np.float16)
    wh[:, 64:64 + FC] = hq.astype(np.float16)

    w1r = inputs["pred_w1"][:V, :].astype(np.float16).reshape(2, 100, FC)
    w2rep = np.tile(inputs["pred_w2"][:, 0].astype(np.float16)[None, :], (P, 1))
    b2rep = np.full((P, 1), inputs["pred_b2"][0], np.float32)
    mv_init = inputs["init_value_memory"].astype(np.float16).reshape(1, -1)
    return dict(ea_table=ea, wh_table=wh, w1r=w1r, w2rep=w2rep, b2rep=b2rep,
                mv_init=mv_init)


def kernel(**inputs):
    inputs = {k: np.asarray(v) for k, v in inputs.items()}
    steps = int(os.environ.get("KERNEL_STEPS", S))

    if steps not in _prog_cache:
        _prog_cache[steps] = _build_program(steps)
    nc = _prog_cache[steps]

    shared = _host_tables(inputs)
    q = inputs["q_data"].astype(np.int64)
    qa = inputs["qa_data"].astype(np.int64)

    in_maps = []
    for core in range(NCORES):
        qs = q[core * BL:(core + 1) * BL]       # [256, S]
        qas = qa[core * BL:(core + 1) * BL]
        # gather order: block g, step k, tile tl, partition p
        #   -> element (g*K + k) of column (tl*128+p)
        def order(x):
            # x [BL, S] -> [S, NT, P] -> [NBLK, KSTEPS, NT, P] flat
            xt = x.T.reshape(S, NT, P)
            return xt.reshape(NBLK, KSTEPS, NT, P).reshape(-1)
        m = dict(shared)
        m["qidx"] = _wrap_idx(order(qs))
        m["qaidx"] = _wrap_idx(order(qas))
        in_maps.append(m)

    trace = bool(int(os.environ.get("KERNEL_TRACE", "0")))
    res = run_bass_kernel_spmd(nc, in_maps, core_ids=list(range(NCORES)), trace=trace)
    global LAST_RESULTS
    LAST_RESULTS = res
    preds = np.concatenate(
        [res.results[i]["preds_out"] for i in range(NCORES)], axis=0
    ).astype(np.float32)
    z = np.zeros_like(preds)
    return (preds, z, z, z)
